# revision 1
# baseline (speedup 1.0000x reference)
"""Trainium2 Bass kernel for BatchRemoveQuatDiscontinuities.

Algorithm (per (batch, joint) lane):
    d[t]    = dot(q[t], q[t-1])                (fp32, 4-wide dot)
    flip[t] = 1 if d[t] < 0 else 0             (t >= 1; flip[0] = 0)
    sigma[t] = (-1)^(sum_{s<=t} flip[s])       (cumulative sign parity)
    out[t]  = q[t] * sigma[t]

Mapping on a NeuronCore (data-parallel over batch across 8 cores):
  * One tile = one batch clip, loaded as a single fully-contiguous 1MB
    DMA: [128 partitions = t/8, free = (ts: 8, j: 64, c: 4)].  This is
    just the flat memory order of q[b], so DMA runs at the HBM roofline.
  * q[t-1]: within a partition it is a free-axis offset (-256); the
    octet boundary (ts=0) needs q[p-1, ts=7], produced by a TensorE
    matmul with an off-diagonal 0/1 matrix S into PSUM (fp32 exact).
  * prod on VectorE, 4-wide dot via two pairwise adds (c0+c1)+(c2+c3),
    written in (j, ts) order; flip indicator e = Relu(Sign(-d)) on
    ScalarE (bf16).
  * Within-octet inclusive prefix: tensor_tensor_scan with a reset mask
    (state = mask*state + e), segments of 8 per joint.  Octet-level
    exclusive prefix: strict-triangular matmul over partitions on the
    per-row totals (strided rhs slice ts=7).  total = rowpref + offs.
  * Parity: cast to int32, &1, then sigma = 1 - 2*p on ScalarE (bf16).
    GpSimd multiplies out = q * sigma (broadcast over c) - exact +/-1.
"""

import numpy as np
import ml_dtypes
from contextlib import ExitStack

import concourse.bass as bass
import concourse.bacc as bacc
import concourse.tile as tile
from concourse import mybir
from concourse.bass_utils import run_bass_kernel_spmd

B, T, J, C = 128, 1024, 64, 4
NCORES = 8
JC = J * C                      # 256 floats per t
BPC = B // NCORES               # 16 batch clips per core
TS = 8                          # t per partition (octet)
FD = TS * JC                    # tile free dim = 2048 floats
SD = J * TS                     # prefix free dim = 512 (j, ts)

FP32 = mybir.dt.float32
BF16 = mybir.dt.bfloat16
I32 = mybir.dt.int32
Alu = mybir.AluOpType
Act = mybir.ActivationFunctionType


def _ap(apx, dims):
    """AP with explicit [step, count] free dims appended to partition dim."""
    return bass.AP(
        tensor=apx.tensor, offset=apx.offset,
        ap=[list(apx.ap[0]), *[list(d) for d in dims]],
    )


def build_nc(bpc=BPC, t=T, reps=1, mode="full", mult_split=4):
    assert t % (128 * TS) == 0
    tpp = t // 128              # t-octets per partition per clip (1 for T=1024)
    nc = bacc.Bacc(None, target_bir_lowering=False)
    q = nc.declare_dram_parameter("q", [bpc, t, J, C], FP32, isOutput=False)
    smat = nc.declare_dram_parameter("smat", [128, 128], FP32, isOutput=False)
    pmat = nc.declare_dram_parameter("pmat", [128, 128], FP32, isOutput=False)
    out = nc.declare_dram_parameter("out", [bpc, t, J, C], FP32, isOutput=True)
    qf = q.rearrange("b t j c -> b (t j c)")
    of = out.rearrange("b t j c -> b (t j c)")

    with tile.TileContext(nc) as tc, ExitStack() as ctx:
        consts = ctx.enter_context(tc.tile_pool(name="consts", bufs=1))
        qpool = ctx.enter_context(tc.tile_pool(name="qpool", bufs=8))
        opool = ctx.enter_context(tc.tile_pool(name="opool", bufs=5))
        spool = ctx.enter_context(tc.tile_pool(name="spool", bufs=4))
        auxp = ctx.enter_context(tc.tile_pool(name="auxp", bufs=4, space="PSUM"))
        offp = ctx.enter_context(tc.tile_pool(name="offp", bufs=4, space="PSUM"))

        smatSB = consts.tile([128, 128], FP32)
        nc.sync.dma_start(out=smatSB[:, :], in_=smat[:, :])
        pmatSB = consts.tile([128, 128], FP32)
        nc.sync.dma_start(out=pmatSB[:, :], in_=pmat[:, :])
        amask = consts.tile([128, SD], FP32)
        nc.vector.memset(amask[:, :], 1.0)
        nc.vector.memset(
            amask.rearrange("p (j ts) -> p j ts", ts=TS)[:, :, 0], 0.0
        )

        def emit_body():
            for b in range(bpc):
                emit_tile(b)

        def emit_tile(b):
            qt = qpool.tile([128, FD], FP32, tag="qt")
            nc.sync.dma_start(
                out=qt[:, :],
                in_=qf[b, :].rearrange("(p x) -> p x", p=128),
            )
            o = opool.tile([128, FD], FP32, tag="o")
            if mode == "dma":
                nc.sync.dma_start(
                    out=of[b, :].rearrange("(p x) -> p x", p=128), in_=qt[:, :]
                )
                return

            # octet-boundary shift: aux[p] = qt[p-1, ts=7 chunk] (row 0 = 0)
            aux = auxp.tile([128, JC], FP32, tag="aux")
            nc.tensor.matmul(
                aux[:, :],
                lhsT=smatSB[:, :],
                rhs=qt[:, FD - JC:FD],
                start=True,
                stop=True,
            )

            # prod: o = q * q_shifted
            nc.vector.tensor_tensor(
                out=o[:, JC:FD], in0=qt[:, JC:FD], in1=qt[:, 0:FD - JC],
                op=Alu.mult,
            )
            nc.vector.tensor_tensor(
                out=o[:, 0:JC], in0=qt[:, 0:JC], in1=aux[:, :], op=Alu.mult,
            )

            # dot over c, pairwise (c0+c1)+(c2+c3); d written in (j, ts) order
            u = spool.tile([128, 2 * SD], FP32, tag="u")
            ov = o.rearrange("p (s c) -> p s c", c=C)
            uv = u.rearrange("p (s k) -> p s k", k=2)
            opairs = ov.rearrange("p s (k two) -> p s k two", k=2)
            nc.vector.tensor_tensor(
                out=uv, in0=opairs[:, :, :, 0], in1=opairs[:, :, :, 1],
                op=Alu.add,
            )
            d = spool.tile([128, SD], FP32, tag="d")  # (j, ts) layout
            u_k = u.rearrange("p (ts j k) -> p ts j k", j=J, k=2)
            nc.vector.tensor_tensor(
                out=_ap(d, [[1, TS], [TS, J]]),
                in0=u_k[:, :, :, 0],
                in1=u_k[:, :, :, 1],
                op=Alu.add,
            )

            # flip indicator e = Relu(Sign(-d)), bf16, (j, ts) layout
            sg = spool.tile([128, SD], FP32, tag="sg")
            nc.scalar.activation(sg[:, :], d[:, :], Act.Sign, scale=-1.0)
            e = spool.tile([128, SD], BF16, tag="e")
            nc.scalar.activation(e[:, :], sg[:, :], Act.Relu)
            # t=0 has no flip (also guards Sign(0) semantics)
            nc.scalar.mul(
                e.rearrange("p (j ts) -> p j ts", ts=TS)[0:1, :, 0],
                e.rearrange("p (j ts) -> p j ts", ts=TS)[0:1, :, 0],
                0.0,
            )

            # within-octet inclusive prefix PARITY (segmented xor-scan):
            # state = (mask * state) xor e  -> 0/1 running parity per joint
            rowp = spool.tile([128, SD], FP32, tag="rowp")
            nc.vector.tensor_tensor_scan(
                out=rowp[:, :], data0=amask[:, :], data1=e[:, :],
                initial=0.0, op0=Alu.mult, op1=Alu.logical_xor,
            )

            # octet-level: count of odd rows above (parity-sum via matmul)
            offs = offp.tile([128, J], FP32, tag="offs")
            nc.tensor.matmul(
                offs[:, :],
                lhsT=pmatSB[:, :],
                rhs=rowp.rearrange("p (j ts) -> p j ts", ts=TS)[:, :, 7],
                start=True,
                stop=True,
            )
            # parity of that count -> sigma_off in {+1, -1} per (p, j)
            offi = spool.tile([128, J], I32, tag="offi")
            nc.vector.tensor_copy(out=offi[:, :], in_=offs[:, :])
            offb = spool.tile([128, J], I32, tag="offb")
            nc.vector.tensor_scalar(
                out=offb[:, :], in0=offi[:, :], scalar1=1, scalar2=None,
                op0=Alu.bitwise_and,
            )
            sigo = spool.tile([128, J], BF16, tag="sigo")
            nc.scalar.activation(sigo[:, :], offb[:, :], Act.Copy,
                                 bias=1.0, scale=-2.0)
            # sigma_row in {+1, -1} from the 0/1 row parity
            sigr = spool.tile([128, SD], BF16, tag="sigr")
            nc.scalar.activation(sigr[:, :], rowp[:, :], Act.Copy,
                                 bias=1.0, scale=-2.0)
            # sigma = sigma_row * sigma_off, (j, ts) layout
            sig = spool.tile([128, SD], BF16, tag="sig")
            nc.vector.tensor_tensor(
                out=sig[:, :], in0=sigr[:, :],
                in1=_ap(sigo, [[1, J], [0, TS]]),
                op=Alu.mult,
            )

            # out = q * sigma (broadcast over c), exact +/-1 multiply;
            # split by ts-range between VectorE and GpSimd
            if mode != "nogp":
                qv = qt.rearrange("p (ts x) -> p ts x", ts=TS)
                ow = o.rearrange("p (ts x) -> p ts x", ts=TS)
                tsplit = mult_split
                if tsplit > 0:
                    nc.vector.tensor_tensor(
                        out=ow[:, 0:tsplit, :],
                        in0=qv[:, 0:tsplit, :],
                        in1=bass.AP(
                            tensor=sig.tensor, offset=sig.offset,
                            ap=[list(sig.ap[0]), [1, tsplit], [TS, J], [0, C]],
                        ),
                        op=Alu.mult,
                    )
                if tsplit < TS:
                    nc.gpsimd.tensor_tensor(
                        out=ow[:, tsplit:TS, :],
                        in0=qv[:, tsplit:TS, :],
                        in1=bass.AP(
                            tensor=sig.tensor, offset=sig.offset + tsplit,
                            ap=[list(sig.ap[0]), [1, TS - tsplit], [TS, J],
                                [0, C]],
                        ),
                        op=Alu.mult,
                    )

            nc.sync.dma_start(
                out=of[b, :].rearrange("(p x) -> p x", p=128), in_=o[:, :]
            )

        if reps == 1:
            emit_body()
        else:
            with tc.For_i(0, reps, 1):
                emit_body()
    return nc


def make_consts():
    smat = np.eye(128, k=1, dtype=np.float32)       # S[k, m] = 1 iff m == k+1
    pmat = np.triu(np.ones((128, 128), np.float32), k=1)  # strict prefix
    return smat, pmat


def kernel(joint_rotations: np.ndarray) -> np.ndarray:
    q = np.ascontiguousarray(joint_rotations, dtype=np.float32)
    assert q.shape == (B, T, J, C)
    smat, pmat = make_consts()
    nc = build_nc()
    nc.finalize()   # run bacc passes (wait splitting, reg alloc) + freeze
    in_maps = [
        {"q": q[c * BPC:(c + 1) * BPC], "smat": smat, "pmat": pmat}
        for c in range(NCORES)
    ]
    res = run_bass_kernel_spmd(nc, in_maps, list(range(NCORES)))
    outs = [np.asarray(r["out"]) for r in res.results]
    return np.concatenate(outs, axis=0)



# revision 3
# speedup vs baseline: 18.2369x; 18.2369x over previous
"""Trainium2 Bass kernel for BatchRemoveQuatDiscontinuities.

Algorithm (per (batch, joint) lane):
    d[t]    = dot(q[t], q[t-1])                (fp32, 4-wide dot)
    flip[t] = 1 if d[t] < 0 else 0             (t >= 1; flip[0] = 0)
    sigma[t] = (-1)^(sum_{s<=t} flip[s])       (cumulative sign parity)
    out[t]  = q[t] * sigma[t]

Mapping on a NeuronCore (data-parallel over batch across 8 cores):
  * One tile = one batch clip, loaded as a single fully-contiguous 1MB
    DMA: [128 partitions = t/8, free = (ts: 8, j: 64, c: 4)].  This is
    just the flat memory order of q[b], so DMA runs at the HBM roofline.
  * q[t-1]: within a partition it is a free-axis offset (-256); the
    octet boundary (ts=0) needs q[p-1, ts=7], produced by a TensorE
    matmul with an off-diagonal 0/1 matrix S into PSUM (fp32 exact).
  * Engine balance (DVE was the bottleneck at ~90% busy): DVE keeps the
    prod multiplies, the strided pair-add into d, and the segmented
    xor-scan; GpSimd (Pool) takes the first pair-add u, the parity
    bit-ops, the sigma combine, and most/all of the final multiply;
    ScalarE does the sign/indicator/copy activations; PE does the two
    shift/prefix matmuls.  All exact +/-1 arithmetic, bit-identical
    to the fp32 reference.
"""

import numpy as np
import ml_dtypes
from contextlib import ExitStack

import concourse.bass as bass
import concourse.bacc as bacc
import concourse.tile as tile
from concourse import mybir
from concourse.bass_utils import run_bass_kernel_spmd

B, T, J, C = 128, 1024, 64, 4
NCORES = 8
JC = J * C                      # 256 floats per t
BPC = B // NCORES               # 16 batch clips per core
TS = 8                          # t per partition (octet)
FD = TS * JC                    # tile free dim = 2048 floats
SD = J * TS                     # prefix free dim = 512 (j, ts)

FP32 = mybir.dt.float32
BF16 = mybir.dt.bfloat16
I32 = mybir.dt.int32
Alu = mybir.AluOpType
Act = mybir.ActivationFunctionType


def _ap(apx, dims):
    """AP with explicit [step, count] free dims appended to partition dim."""
    return bass.AP(
        tensor=apx.tensor, offset=apx.offset,
        ap=[list(apx.ap[0]), *[list(d) for d in dims]],
    )


def build_nc(bpc=BPC, t=T, reps=1, mode="full", mult_split=0, u_eng="gpsimd",
             out_q="sync"):
    assert t % (128 * TS) == 0
    nc = bacc.Bacc(None, target_bir_lowering=False)
    q = nc.declare_dram_parameter("q", [bpc, t, J, C], FP32, isOutput=False)
    smat = nc.declare_dram_parameter("smat", [128, 128], FP32, isOutput=False)
    pmat = nc.declare_dram_parameter("pmat", [128, 128], FP32, isOutput=False)
    out = nc.declare_dram_parameter("out", [bpc, t, J, C], FP32, isOutput=True)
    qf = q.rearrange("b t j c -> b (t j c)")
    of = out.rearrange("b t j c -> b (t j c)")

    with tile.TileContext(nc) as tc, ExitStack() as ctx:
        consts = ctx.enter_context(tc.tile_pool(name="consts", bufs=1))
        qpool = ctx.enter_context(tc.tile_pool(name="qpool", bufs=8))
        opool = ctx.enter_context(tc.tile_pool(name="opool", bufs=5))
        spool = ctx.enter_context(tc.tile_pool(name="spool", bufs=4))
        auxp = ctx.enter_context(tc.tile_pool(name="auxp", bufs=4, space="PSUM"))
        offp = ctx.enter_context(tc.tile_pool(name="offp", bufs=4, space="PSUM"))

        smatSB = consts.tile([128, 128], FP32)
        nc.sync.dma_start(out=smatSB[:, :], in_=smat[:, :])
        pmatSB = consts.tile([128, 128], FP32)
        nc.sync.dma_start(out=pmatSB[:, :], in_=pmat[:, :])
        amask = consts.tile([128, SD], FP32)
        nc.vector.memset(amask[:, :], 1.0)
        nc.vector.memset(
            amask.rearrange("p (j ts) -> p j ts", ts=TS)[:, :, 0], 0.0
        )

        u_e = getattr(nc, u_eng)
        out_e = getattr(nc, out_q)

        def emit_body():
            for b in range(bpc):
                emit_tile(b)

        def emit_tile(b):
            qt = qpool.tile([128, FD], FP32, tag="qt")
            nc.sync.dma_start(
                out=qt[:, :],
                in_=qf[b, :].rearrange("(p x) -> p x", p=128),
            )
            o = opool.tile([128, FD], FP32, tag="o")
            if mode == "dma":
                nc.sync.dma_start(
                    out=of[b, :].rearrange("(p x) -> p x", p=128), in_=qt[:, :]
                )
                return

            # octet-boundary shift: aux[p] = qt[p-1, ts=7 chunk] (row 0 = 0)
            aux = auxp.tile([128, JC], FP32, tag="aux")
            nc.tensor.matmul(
                aux[:, :],
                lhsT=smatSB[:, :],
                rhs=qt[:, FD - JC:FD],
                start=True,
                stop=True,
            )

            # prod: o = q * q_shifted  (DVE; the low chunk reads PSUM)
            nc.vector.tensor_tensor(
                out=o[:, JC:FD], in0=qt[:, JC:FD], in1=qt[:, 0:FD - JC],
                op=Alu.mult,
            )
            nc.vector.tensor_tensor(
                out=o[:, 0:JC], in0=qt[:, 0:JC], in1=aux[:, :], op=Alu.mult,
            )

            # dot over c, pairwise (c0+c1)+(c2+c3); d written in (j, ts) order
            u = spool.tile([128, 2 * SD], FP32, tag="u")
            ov = o.rearrange("p (s c) -> p s c", c=C)
            uv = u.rearrange("p (s k) -> p s k", k=2)
            opairs = ov.rearrange("p s (k two) -> p s k two", k=2)
            u_e.tensor_tensor(
                out=uv, in0=opairs[:, :, :, 0], in1=opairs[:, :, :, 1],
                op=Alu.add,
            )
            d = spool.tile([128, SD], FP32, tag="d")  # (j, ts) layout
            u_k = u.rearrange("p (ts j k) -> p ts j k", j=J, k=2)
            nc.vector.tensor_tensor(
                out=_ap(d, [[1, TS], [TS, J]]),
                in0=u_k[:, :, :, 0],
                in1=u_k[:, :, :, 1],
                op=Alu.add,
            )

            # flip indicator e = Relu(Sign(-d)), bf16, (j, ts) layout
            sg = spool.tile([128, SD], FP32, tag="sg")
            nc.scalar.activation(sg[:, :], d[:, :], Act.Sign, scale=-1.0)
            e = spool.tile([128, SD], BF16, tag="e")
            nc.scalar.activation(e[:, :], sg[:, :], Act.Relu)
            # t=0 has no flip (also guards Sign(0) semantics)
            nc.scalar.mul(
                e.rearrange("p (j ts) -> p j ts", ts=TS)[0:1, :, 0],
                e.rearrange("p (j ts) -> p j ts", ts=TS)[0:1, :, 0],
                0.0,
            )

            # within-octet inclusive prefix PARITY (segmented xor-scan):
            # state = (mask * state) xor e  -> 0/1 running parity per joint
            rowp = spool.tile([128, SD], FP32, tag="rowp")
            nc.vector.tensor_tensor_scan(
                out=rowp[:, :], data0=amask[:, :], data1=e[:, :],
                initial=0.0, op0=Alu.mult, op1=Alu.logical_xor,
            )

            # octet-level: count of odd rows above (parity-sum via matmul)
            offs = offp.tile([128, J], FP32, tag="offs")
            nc.tensor.matmul(
                offs[:, :],
                lhsT=pmatSB[:, :],
                rhs=rowp.rearrange("p (j ts) -> p j ts", ts=TS)[:, :, 7],
                start=True,
                stop=True,
            )
            # parity of that count -> sigma_off in {+1, -1} per (p, j)
            # (int copy reads PSUM, stays on DVE; bit-and runs on GpSimd)
            offi = spool.tile([128, J], I32, tag="offi")
            nc.vector.tensor_copy(out=offi[:, :], in_=offs[:, :])
            offb = spool.tile([128, J], I32, tag="offb")
            nc.vector.tensor_scalar(
                out=offb[:, :], in0=offi[:, :], scalar1=1, scalar2=None,
                op0=Alu.bitwise_and,
            )
            sigo = spool.tile([128, J], BF16, tag="sigo")
            nc.scalar.activation(sigo[:, :], offb[:, :], Act.Copy,
                                 bias=1.0, scale=-2.0)
            # sigma_row in {+1, -1} from the 0/1 row parity
            sigr = spool.tile([128, SD], BF16, tag="sigr")
            nc.scalar.activation(sigr[:, :], rowp[:, :], Act.Copy,
                                 bias=1.0, scale=-2.0)
            # sigma = sigma_row * sigma_off, (j, ts) layout
            sig = spool.tile([128, SD], BF16, tag="sig")
            nc.gpsimd.tensor_tensor(
                out=sig[:, :], in0=sigr[:, :],
                in1=_ap(sigo, [[1, J], [0, TS]]),
                op=Alu.mult,
            )

            # out = q * sigma (broadcast over c), exact +/-1 multiply;
            # split by ts-range between VectorE and GpSimd
            if mode != "nogp":
                qv = qt.rearrange("p (ts x) -> p ts x", ts=TS)
                ow = o.rearrange("p (ts x) -> p ts x", ts=TS)
                tsplit = mult_split
                if tsplit > 0:
                    nc.vector.tensor_tensor(
                        out=ow[:, 0:tsplit, :],
                        in0=qv[:, 0:tsplit, :],
                        in1=bass.AP(
                            tensor=sig.tensor, offset=sig.offset,
                            ap=[list(sig.ap[0]), [1, tsplit], [TS, J], [0, C]],
                        ),
                        op=Alu.mult,
                    )
                if tsplit < TS:
                    nc.gpsimd.tensor_tensor(
                        out=ow[:, tsplit:TS, :],
                        in0=qv[:, tsplit:TS, :],
                        in1=bass.AP(
                            tensor=sig.tensor, offset=sig.offset + tsplit,
                            ap=[list(sig.ap[0]), [1, TS - tsplit], [TS, J],
                                [0, C]],
                        ),
                        op=Alu.mult,
                    )

            out_e.dma_start(
                out=of[b, :].rearrange("(p x) -> p x", p=128), in_=o[:, :]
            )

        if reps == 1:
            emit_body()
        else:
            with tc.For_i(0, reps, 1):
                emit_body()
    return nc


def make_consts():
    smat = np.eye(128, k=1, dtype=np.float32)       # S[k, m] = 1 iff m == k+1
    pmat = np.triu(np.ones((128, 128), np.float32), k=1)  # strict prefix
    return smat, pmat


def kernel(joint_rotations: np.ndarray) -> np.ndarray:
    q = np.ascontiguousarray(joint_rotations, dtype=np.float32)
    assert q.shape == (B, T, J, C)
    smat, pmat = make_consts()
    nc = build_nc()
    nc.finalize()   # run bacc passes (wait splitting, reg alloc) + freeze
    in_maps = [
        {"q": q[c * BPC:(c + 1) * BPC], "smat": smat, "pmat": pmat}
        for c in range(NCORES)
    ]
    res = run_bass_kernel_spmd(nc, in_maps, list(range(NCORES)))
    outs = [np.asarray(r["out"]) for r in res.results]
    return np.concatenate(outs, axis=0)


# revision 16
# speedup vs baseline: 21.4540x; 1.1764x over previous
"""Trainium2 Bass kernel for BatchRemoveQuatDiscontinuities.

Algorithm (per (batch, joint) lane):
    d[t]    = dot(q[t], q[t-1])                (fp32, 4-wide dot)
    flip[t] = 1 if d[t] < 0 else 0             (t >= 1; flip[0] = 0)
    sigma[t] = (-1)^(sum_{s<=t} flip[s])       (cumulative sign parity)
    out[t]  = q[t] * sigma[t]

Mapping on a NeuronCore (data-parallel over batch across 8 cores):
  * One tile = G batch clips ("gclips"), loaded as a single DMA:
    [128 partitions = t/8, free = (b: G, ts: 8, j: 64, c: 4)].  Each
    partition line is G contiguous-8KB spans, so DMA runs at the HBM
    roofline; G > 1 halves the instruction / semaphore count per clip
    (the kernel is latency- not throughput-bound).
  * q[t-1]: within a partition a free-axis offset (-256); the octet
    boundary (ts=0) comes from a TensorE shift-matmul into PSUM.
  * d via two pairwise adds; flip indicator e = Relu(Sign(-d)) on
    ScalarE (bf16); segmented xor-scan gives within-octet parity rowp;
    a strict-upper-triangular matmul counts odd octets above; parity
    of the count via int32 cast + bitwise-and; sigma = (4*rowp - 2) *
    (parity - 0.5) in {+/-1} exactly.
  * out = q * sigma (broadcast over c), exact +/-1 multiply, split by
    ts-range between VectorE and GpSimd (mult_split = DVE share of 8).
"""

import numpy as np
import ml_dtypes
from contextlib import ExitStack

import concourse.bass as bass
import concourse.bacc as bacc
import concourse.tile as tile
from concourse import mybir
from concourse.bass_utils import run_bass_kernel_spmd

B, T, J, C = 128, 1024, 64, 4
NCORES = 8
JC = J * C                      # 256 floats per t
BPC = B // NCORES               # 16 batch clips per core
TS = 8                          # t per partition (octet)
FD = TS * JC                    # per-clip free dim = 2048 floats
SD = J * TS                     # per-clip prefix free dim = 512 (j, ts)

FP32 = mybir.dt.float32
BF16 = mybir.dt.bfloat16
I32 = mybir.dt.int32
Alu = mybir.AluOpType
Act = mybir.ActivationFunctionType


def _ap(apx, dims, extra_offset=0):
    """AP with explicit [step, count] free dims appended to partition dim."""
    return bass.AP(
        tensor=apx.tensor, offset=apx.offset + extra_offset,
        ap=[list(apx.ap[0]), *[list(d) for d in dims]],
    )


def build_nc(bpc=BPC, t=T, reps=1, mode="full", mult_split=4,
             u_eng="vector", sig_eng="vector", out_q="sync", in_q="scalar",
             gclips=2, qbufs=4, obufs=3, sbufs=3, stage=99):
    assert t % (128 * TS) == 0
    G = gclips
    assert bpc % G == 0
    nc = bacc.Bacc(None, target_bir_lowering=False)
    q = nc.declare_dram_parameter("q", [bpc, t, J, C], FP32, isOutput=False)
    smat = nc.declare_dram_parameter("smat", [128, 128], FP32, isOutput=False)
    pmat = nc.declare_dram_parameter("pmat", [128, 128], FP32, isOutput=False)
    out = nc.declare_dram_parameter("out", [bpc, t, J, C], FP32, isOutput=True)
    qf = q.rearrange("b t j c -> b (t j c)")
    of = out.rearrange("b t j c -> b (t j c)")

    with tile.TileContext(nc) as tc, ExitStack() as ctx:
        consts = ctx.enter_context(tc.tile_pool(name="consts", bufs=1))
        qpool = ctx.enter_context(tc.tile_pool(name="qpool", bufs=qbufs))
        opool = ctx.enter_context(tc.tile_pool(name="opool", bufs=obufs))
        spool = ctx.enter_context(tc.tile_pool(name="spool", bufs=sbufs))
        auxp = ctx.enter_context(tc.tile_pool(name="auxp", bufs=4, space="PSUM"))
        offp = ctx.enter_context(tc.tile_pool(name="offp", bufs=4, space="PSUM"))

        smatSB = consts.tile([128, 128], FP32)
        nc.sync.dma_start(out=smatSB[:, :], in_=smat[:, :])
        pmatSB = consts.tile([128, 128], FP32)
        nc.sync.dma_start(out=pmatSB[:, :], in_=pmat[:, :])
        amask = consts.tile([128, G * SD], FP32)
        nc.vector.memset(amask[:, :], 1.0)
        nc.vector.memset(
            amask.rearrange("p (g j ts) -> p (g j) ts", j=J, ts=TS)[:, :, 0], 0.0
        )

        u_e = getattr(nc, u_eng)
        sig_e = getattr(nc, sig_eng)
        out_e = getattr(nc, out_q)
        in_e = getattr(nc, in_q)

        def emit_body():
            for g in range(bpc // G):
                emit_tile(g)

        def emit_tile(g):
            b0 = g * G
            qt = qpool.tile([128, G * FD], FP32, tag="qt")
            in_e.dma_start(
                out=qt[:, :],
                in_=qf[b0:b0 + G, :].rearrange("b (p x) -> p b x", p=128),
            )
            o = opool.tile([128, G * FD], FP32, tag="o")
            if mode == "dma":
                out_e.dma_start(
                    out=of[b0:b0 + G, :].rearrange("b (p x) -> p b x", p=128),
                    in_=qt[:, :],
                )
                return
            if mode.startswith("bench"):
                # mode = "bench:<eng>:<elems>:<count>" — rate microbench
                _, eng, elems, count = mode.split(":")
                elems, count = int(elems), int(count)
                be = getattr(nc, eng)
                for i in range(count):
                    if eng == "scalar":
                        be.activation(o[:, 0:elems], qt[:, 0:elems], Act.Sign,
                                      scale=-1.0)
                    else:
                        be.tensor_tensor(out=o[:, 0:elems], in0=qt[:, 0:elems],
                                         in1=qt[:, G * FD - elems:G * FD],
                                         op=Alu.mult)
                out_e.dma_start(
                    out=of[b0:b0 + G, :].rearrange("b (p x) -> p b x", p=128),
                    in_=o[:, :],
                )
                return
            if mode.startswith("bcast"):
                # mode = "bcast:<eng>:<count>" — broadcast-read microbench:
                # out 2048 contiguous = qt 2048 * qt[(ts,j)] bcast over C
                _, eng, count = mode.split(":")
                be = getattr(nc, eng)
                bc = _ap(qt, [[1, TS], [TS, J], [0, C]])
                for i in range(int(count)):
                    if eng == "scalar":
                        be.activation(o[:, 0:FD], bc, Act.Copy)
                    else:
                        be.tensor_tensor(out=o[:, 0:FD], in0=qt[:, 0:FD],
                                         in1=bc, op=Alu.mult)
                out_e.dma_start(
                    out=of[b0:b0 + G, :].rearrange("b (p x) -> p b x", p=128),
                    in_=o[:, :],
                )
                return

            # octet-boundary shift: aux[p, (b, jc)] = qt[p-1, b, ts=7 chunk]
            aux = auxp.tile([128, G * JC], FP32, tag="aux")
            nc.tensor.matmul(
                aux[:, :],
                lhsT=smatSB[:, :],
                rhs=_ap(qt, [[FD, G], [1, JC]], extra_offset=FD - JC),
                start=True,
                stop=True,
            )

            # prod: o = q * q_shifted  (DVE; the low chunk reads PSUM)
            nc.vector.tensor_tensor(
                out=_ap(o, [[FD, G], [1, FD - JC]], extra_offset=JC),
                in0=_ap(qt, [[FD, G], [1, FD - JC]], extra_offset=JC),
                in1=_ap(qt, [[FD, G], [1, FD - JC]]),
                op=Alu.mult,
            )
            nc.vector.tensor_tensor(
                out=_ap(o, [[FD, G], [1, JC]]),
                in0=_ap(qt, [[FD, G], [1, JC]]),
                in1=aux[:, :],
                op=Alu.mult,
            )

            if stage < 2:
                out_e.dma_start(
                    out=of[b0:b0 + G, :].rearrange("b (p x) -> p b x", p=128),
                    in_=o[:, :],
                )
                return
            # pairwise sums u = (c0+c1, c2+c3), (b, ts*j, k) order
            u = spool.tile([128, G * 2 * SD], FP32, tag="u")
            u_e.tensor_tensor(
                out=_ap(u, [[2 * SD, G], [2, SD], [1, 2]]),
                in0=_ap(o, [[FD, G], [4, SD], [2, 2]]),
                in1=_ap(o, [[FD, G], [4, SD], [2, 2]], extra_offset=1),
                op=Alu.add,
            )
            # d = u0 + u1, written in (b, j, ts) order for the scan
            d = spool.tile([128, G * SD], FP32, tag="d")
            nc.vector.tensor_tensor(
                out=_ap(d, [[SD, G], [1, TS], [TS, J]]),
                in0=_ap(u, [[2 * SD, G], [2 * J, TS], [2, J]]),
                in1=_ap(u, [[2 * SD, G], [2 * J, TS], [2, J]], extra_offset=1),
                op=Alu.add,
            )
            if stage < 3:
                out_e.dma_start(
                    out=of[b0:b0 + G, :].rearrange("b (p x) -> p b x", p=128),
                    in_=o[:, :],
                )
                return
            # flip indicator e = Relu(Sign(-d)), bf16
            sg = spool.tile([128, G * SD], FP32, tag="sg")
            nc.scalar.activation(sg[:, :], d[:, :], Act.Sign, scale=-1.0)
            e = spool.tile([128, G * SD], BF16, tag="e")
            nc.scalar.activation(e[:, :], sg[:, :], Act.Relu)
            # t=0 has no flip (also guards Sign(0) semantics)
            nc.scalar.mul(
                _ap(e[0:1, :], [[TS, G * J]]),
                _ap(e[0:1, :], [[TS, G * J]]),
                0.0,
            )

            if stage < 4:
                out_e.dma_start(
                    out=of[b0:b0 + G, :].rearrange("b (p x) -> p b x", p=128),
                    in_=o[:, :],
                )
                return
            # within-octet inclusive prefix PARITY (segmented xor-scan):
            # state = (mask * state) xor e  -> 0/1 running parity per joint
            rowp = spool.tile([128, G * SD], FP32, tag="rowp")
            nc.vector.tensor_tensor_scan(
                out=rowp[:, :], data0=amask[:, :], data1=e[:, :],
                initial=0.0, op0=Alu.mult, op1=Alu.logical_xor,
            )

            if stage < 5:
                out_e.dma_start(
                    out=of[b0:b0 + G, :].rearrange("b (p x) -> p b x", p=128),
                    in_=o[:, :],
                )
                return
            # octet-level: count of odd rows above (parity-sum via matmul)
            offs = offp.tile([128, G * J], FP32, tag="offs")
            nc.tensor.matmul(
                offs[:, :],
                lhsT=pmatSB[:, :],
                rhs=_ap(rowp, [[SD, G], [TS, J]], extra_offset=TS - 1),
                start=True,
                stop=True,
            )
            # parity of the count: int cast + bitwise and
            offi = spool.tile([128, G * J], I32, tag="offi")
            nc.vector.tensor_copy(out=offi[:, :], in_=offs[:, :])
            offb = spool.tile([128, G * J], I32, tag="offb")
            nc.vector.tensor_scalar(
                out=offb[:, :], in0=offi[:, :], scalar1=1, scalar2=None,
                op0=Alu.bitwise_and,
            )
            # offh = parity - 0.5 in {-0.5, +0.5}
            offh = spool.tile([128, G * J], BF16, tag="offh")
            nc.scalar.activation(offh[:, :], offb[:, :], Act.Copy,
                                 bias=-0.5, scale=1.0)
            # sigma_row4 = 4*rowp - 2 in {-2, +2}
            sigr = spool.tile([128, G * SD], BF16, tag="sigr")
            nc.scalar.activation(sigr[:, :], rowp[:, :], Act.Copy,
                                 bias=-2.0, scale=4.0)
            # sigma = sigr4 * offh in {+1, -1}, (b, j, ts) layout
            sig = spool.tile([128, G * SD], BF16, tag="sig")
            sig_e.tensor_tensor(
                out=sig[:, :],
                in0=sigr[:, :],
                in1=_ap(offh, [[J, G], [1, J], [0, TS]]),
                op=Alu.mult,
            )

            if stage < 6:
                out_e.dma_start(
                    out=of[b0:b0 + G, :].rearrange("b (p x) -> p b x", p=128),
                    in_=o[:, :],
                )
                return
            # out = q * sigma (broadcast over c), exact +/-1 multiply;
            # split by ts-range between VectorE and GpSimd, per clip
            tsplit = mult_split
            for b in range(G):
                if tsplit > 0:
                    nc.vector.tensor_tensor(
                        out=_ap(o, [[1, tsplit * JC]], extra_offset=b * FD),
                        in0=_ap(qt, [[1, tsplit * JC]], extra_offset=b * FD),
                        in1=_ap(sig, [[1, tsplit], [TS, J], [0, C]],
                                extra_offset=b * SD),
                        op=Alu.mult,
                    )
                if tsplit < TS:
                    nc.gpsimd.tensor_tensor(
                        out=_ap(o, [[1, (TS - tsplit) * JC]],
                                extra_offset=b * FD + tsplit * JC),
                        in0=_ap(qt, [[1, (TS - tsplit) * JC]],
                                extra_offset=b * FD + tsplit * JC),
                        in1=_ap(sig, [[1, TS - tsplit], [TS, J], [0, C]],
                                extra_offset=b * SD + tsplit),
                        op=Alu.mult,
                    )

            out_e.dma_start(
                out=of[b0:b0 + G, :].rearrange("b (p x) -> p b x", p=128),
                in_=o[:, :],
            )

        if reps == 1:
            emit_body()
        else:
            with tc.For_i(0, reps, 1):
                emit_body()
    return nc


def make_consts():
    smat = np.eye(128, k=1, dtype=np.float32)       # S[k, m] = 1 iff m == k+1
    pmat = np.triu(np.ones((128, 128), np.float32), k=1)  # strict prefix
    return smat, pmat


def kernel(joint_rotations: np.ndarray) -> np.ndarray:
    q = np.ascontiguousarray(joint_rotations, dtype=np.float32)
    assert q.shape == (B, T, J, C)
    smat, pmat = make_consts()
    nc = build_nc()
    nc.finalize()   # run bacc passes (wait splitting, reg alloc) + freeze
    in_maps = [
        {"q": q[c * BPC:(c + 1) * BPC], "smat": smat, "pmat": pmat}
        for c in range(NCORES)
    ]
    res = run_bass_kernel_spmd(nc, in_maps, list(range(NCORES)))
    outs = [np.asarray(r["out"]) for r in res.results]
    return np.concatenate(outs, axis=0)


# revision 21
# speedup vs baseline: 23.4201x; 1.0916x over previous
"""Trainium2 Bass kernel for BatchRemoveQuatDiscontinuities.

Algorithm (per (batch, joint) lane):
    d[t]    = dot(q[t], q[t-1])                (fp32, 4-wide dot)
    flip[t] = 1 if d[t] < 0 else 0             (t >= 1; flip[0] = 0)
    sigma[t] = (-1)^(sum_{s<=t} flip[s])       (cumulative sign parity)
    out[t]  = q[t] * sigma[t]

Mapping on a NeuronCore (data-parallel over batch across 8 cores):
  * One tile = G batch clips ("gclips"), loaded as a single DMA:
    [128 partitions = t/8, free = (b: G, ts: 8, j: 64, c: 4)].  Each
    partition line is G contiguous-8KB spans, so DMA runs at the HBM
    roofline; G > 1 halves the instruction / semaphore count per clip
    (the kernel is latency- not throughput-bound).
  * q[t-1]: within a partition a free-axis offset (-256); the octet
    boundary (ts=0) comes from a TensorE shift-matmul into PSUM.
  * d via two pairwise adds; flip indicator e = Relu(Sign(-d)) on
    ScalarE (bf16); segmented xor-scan gives within-octet parity rowp;
    a strict-upper-triangular matmul counts odd octets above; parity
    of the count via int32 cast + bitwise-and; sigma = (4*rowp - 2) *
    (parity - 0.5) in {+/-1} exactly.
  * out = q * sigma (broadcast over c), exact +/-1 multiply on VectorE
    (GpSimd measured ~2.6 ns/elem vs DVE 1.16 on real HW and poisons the
    chain tail; mult_split=8 keeps it off the pipeline entirely).
  * Software-pipelined emission (prefetch=4): the in-DMA for chunk g+4
    is issued before chunk g's compute, so the SP DMA FIFO never
    head-of-line blocks input loads behind an out-DMA waiting on the
    compute chain.  Per-clip out-DMAs (out_split) let the first MB
    stream out while the second clip's multiply finishes.
"""

import numpy as np
import ml_dtypes
from contextlib import ExitStack

import concourse.bass as bass
import concourse.bacc as bacc
import concourse.tile as tile
from concourse import mybir
from concourse.bass_utils import run_bass_kernel_spmd

B, T, J, C = 128, 1024, 64, 4
NCORES = 8
JC = J * C                      # 256 floats per t
BPC = B // NCORES               # 16 batch clips per core
TS = 8                          # t per partition (octet)
FD = TS * JC                    # per-clip free dim = 2048 floats
SD = J * TS                     # per-clip prefix free dim = 512 (j, ts)

FP32 = mybir.dt.float32
BF16 = mybir.dt.bfloat16
I32 = mybir.dt.int32
Alu = mybir.AluOpType
Act = mybir.ActivationFunctionType


def _ap(apx, dims, extra_offset=0):
    """AP with explicit [step, count] free dims appended to partition dim."""
    return bass.AP(
        tensor=apx.tensor, offset=apx.offset + extra_offset,
        ap=[list(apx.ap[0]), *[list(d) for d in dims]],
    )


def build_nc(bpc=BPC, t=T, reps=1, mode="full", mult_split=8,
             u_eng="vector", sig_eng="vector", out_q="sync", in_q="sync",
             gclips=2, qbufs=6, obufs=3, sbufs=2, stage=99, prefetch=4,
             out_split=True, fixup=False, fuse_parity=False):
    assert t % (128 * TS) == 0
    G = gclips
    assert bpc % G == 0
    nc = bacc.Bacc(None, target_bir_lowering=False)
    q = nc.declare_dram_parameter("q", [bpc, t, J, C], FP32, isOutput=False)
    smat = nc.declare_dram_parameter("smat", [128, 128], FP32, isOutput=False)
    pmat = nc.declare_dram_parameter("pmat", [128, 128], FP32, isOutput=False)
    out = nc.declare_dram_parameter("out", [bpc, t, J, C], FP32, isOutput=True)
    qf = q.rearrange("b t j c -> b (t j c)")
    of = out.rearrange("b t j c -> b (t j c)")

    with tile.TileContext(nc) as tc, ExitStack() as ctx:
        consts = ctx.enter_context(tc.tile_pool(name="consts", bufs=1))
        qpool = ctx.enter_context(tc.tile_pool(name="qpool", bufs=qbufs))
        opool = ctx.enter_context(tc.tile_pool(name="opool", bufs=obufs))
        spool = ctx.enter_context(tc.tile_pool(name="spool", bufs=sbufs))
        auxp = ctx.enter_context(tc.tile_pool(name="auxp", bufs=4, space="PSUM"))
        offp = ctx.enter_context(tc.tile_pool(name="offp", bufs=4, space="PSUM"))

        smatSB = consts.tile([128, 128], FP32)
        nc.sync.dma_start(out=smatSB[:, :], in_=smat[:, :])
        pmatSB = consts.tile([128, 128], FP32)
        nc.sync.dma_start(out=pmatSB[:, :], in_=pmat[:, :])
        amask = consts.tile([128, G * SD], FP32)
        nc.vector.memset(amask[:, :], 1.0)
        nc.vector.memset(
            amask.rearrange("p (g j ts) -> p (g j) ts", j=J, ts=TS)[:, :, 0], 0.0
        )

        u_e = getattr(nc, u_eng)
        sig_e = getattr(nc, sig_eng)
        out_e = getattr(nc, out_q)
        in_e = getattr(nc, in_q)

        nchunks = bpc // G
        qts = {}

        def emit_load(g):
            b0 = g * G
            qts[g] = qpool.tile([128, G * FD], FP32, tag="qt", name=f"qt{g}")
            in_e.dma_start(
                out=qts[g][:, :],
                in_=qf[b0:b0 + G, :].rearrange("b (p x) -> p b x", p=128),
            )

        def emit_body():
            pf = min(prefetch, nchunks - 1)
            for g in range(pf):
                emit_load(g)
            for g in range(nchunks):
                if g + pf < nchunks:
                    emit_load(g + pf)
                emit_tile(g)

        def emit_tile(g):
            b0 = g * G
            qt = qts.pop(g)
            o = opool.tile([128, G * FD], FP32, tag="o")
            if mode == "dma":
                out_e.dma_start(
                    out=of[b0:b0 + G, :].rearrange("b (p x) -> p b x", p=128),
                    in_=qt[:, :],
                )
                return
            if mode.startswith("bench"):
                # mode = "bench:<eng>:<elems>:<count>" — rate microbench
                _, eng, elems, count = mode.split(":")
                elems, count = int(elems), int(count)
                be = getattr(nc, eng)
                for i in range(count):
                    if eng == "scalar":
                        be.activation(o[:, 0:elems], qt[:, 0:elems], Act.Sign,
                                      scale=-1.0)
                    else:
                        be.tensor_tensor(out=o[:, 0:elems], in0=qt[:, 0:elems],
                                         in1=qt[:, G * FD - elems:G * FD],
                                         op=Alu.mult)
                out_e.dma_start(
                    out=of[b0:b0 + G, :].rearrange("b (p x) -> p b x", p=128),
                    in_=o[:, :],
                )
                return
            if mode.startswith("bcast"):
                # mode = "bcast:<eng>:<count>" — broadcast-read microbench:
                # out 2048 contiguous = qt 2048 * qt[(ts,j)] bcast over C
                _, eng, count = mode.split(":")
                be = getattr(nc, eng)
                bc = _ap(qt, [[1, TS], [TS, J], [0, C]])
                for i in range(int(count)):
                    if eng == "scalar":
                        be.activation(o[:, 0:FD], bc, Act.Copy)
                    else:
                        be.tensor_tensor(out=o[:, 0:FD], in0=qt[:, 0:FD],
                                         in1=bc, op=Alu.mult)
                out_e.dma_start(
                    out=of[b0:b0 + G, :].rearrange("b (p x) -> p b x", p=128),
                    in_=o[:, :],
                )
                return

            # octet-boundary shift: aux[p, (b, jc)] = qt[p-1, b, ts=7 chunk]
            aux = auxp.tile([128, G * JC], FP32, tag="aux")
            nc.tensor.matmul(
                aux[:, :],
                lhsT=smatSB[:, :],
                rhs=_ap(qt, [[FD, G], [1, JC]], extra_offset=FD - JC),
                start=True,
                stop=True,
            )

            # prod: o = q * q_shifted  (DVE; the low chunk reads PSUM)
            nc.vector.tensor_tensor(
                out=_ap(o, [[FD, G], [1, FD - JC]], extra_offset=JC),
                in0=_ap(qt, [[FD, G], [1, FD - JC]], extra_offset=JC),
                in1=_ap(qt, [[FD, G], [1, FD - JC]]),
                op=Alu.mult,
            )
            nc.vector.tensor_tensor(
                out=_ap(o, [[FD, G], [1, JC]]),
                in0=_ap(qt, [[FD, G], [1, JC]]),
                in1=aux[:, :],
                op=Alu.mult,
            )

            if stage < 2:
                out_e.dma_start(
                    out=of[b0:b0 + G, :].rearrange("b (p x) -> p b x", p=128),
                    in_=o[:, :],
                )
                return
            # pairwise sums u = (c0+c1, c2+c3), (b, ts*j, k) order
            u = spool.tile([128, G * 2 * SD], FP32, tag="u")
            u_e.tensor_tensor(
                out=_ap(u, [[2 * SD, G], [2, SD], [1, 2]]),
                in0=_ap(o, [[FD, G], [4, SD], [2, 2]]),
                in1=_ap(o, [[FD, G], [4, SD], [2, 2]], extra_offset=1),
                op=Alu.add,
            )
            # d = u0 + u1, written in (b, j, ts) order for the scan
            d = spool.tile([128, G * SD], FP32, tag="d")
            nc.vector.tensor_tensor(
                out=_ap(d, [[SD, G], [1, TS], [TS, J]]),
                in0=_ap(u, [[2 * SD, G], [2 * J, TS], [2, J]]),
                in1=_ap(u, [[2 * SD, G], [2 * J, TS], [2, J]], extra_offset=1),
                op=Alu.add,
            )
            if stage < 3:
                out_e.dma_start(
                    out=of[b0:b0 + G, :].rearrange("b (p x) -> p b x", p=128),
                    in_=o[:, :],
                )
                return
            # flip indicator e = Relu(Sign(-d)), bf16
            sg = spool.tile([128, G * SD], FP32, tag="sg")
            nc.scalar.activation(sg[:, :], d[:, :], Act.Sign, scale=-1.0)
            e = spool.tile([128, G * SD], BF16, tag="e")
            nc.scalar.activation(e[:, :], sg[:, :], Act.Relu)
            if fixup:
                # t=0 has no flip (also guards Sign(0) semantics)
                nc.scalar.mul(
                    _ap(e[0:1, :], [[TS, G * J]]),
                    _ap(e[0:1, :], [[TS, G * J]]),
                    0.0,
                )

            if stage < 4:
                out_e.dma_start(
                    out=of[b0:b0 + G, :].rearrange("b (p x) -> p b x", p=128),
                    in_=o[:, :],
                )
                return
            # within-octet inclusive prefix PARITY (segmented xor-scan):
            # state = (mask * state) xor e  -> 0/1 running parity per joint
            rowp = spool.tile([128, G * SD], FP32, tag="rowp")
            nc.vector.tensor_tensor_scan(
                out=rowp[:, :], data0=amask[:, :], data1=e[:, :],
                initial=0.0, op0=Alu.mult, op1=Alu.logical_xor,
            )

            if stage < 5:
                out_e.dma_start(
                    out=of[b0:b0 + G, :].rearrange("b (p x) -> p b x", p=128),
                    in_=o[:, :],
                )
                return
            # octet-level: count of odd rows above (parity-sum via matmul)
            offs = offp.tile([128, G * J], FP32, tag="offs")
            nc.tensor.matmul(
                offs[:, :],
                lhsT=pmatSB[:, :],
                rhs=_ap(rowp, [[SD, G], [TS, J]], extra_offset=TS - 1),
                start=True,
                stop=True,
            )
            # parity of the count: int cast + bitwise and
            offb = spool.tile([128, G * J], I32, tag="offb")
            if fuse_parity:
                nc.vector.tensor_scalar(
                    out=offb[:, :], in0=offs[:, :], scalar1=1, scalar2=None,
                    op0=Alu.bitwise_and,
                )
            else:
                offi = spool.tile([128, G * J], I32, tag="offi")
                nc.vector.tensor_copy(out=offi[:, :], in_=offs[:, :])
                nc.vector.tensor_scalar(
                    out=offb[:, :], in0=offi[:, :], scalar1=1, scalar2=None,
                    op0=Alu.bitwise_and,
                )
            # offh = parity - 0.5 in {-0.5, +0.5}
            offh = spool.tile([128, G * J], BF16, tag="offh")
            nc.scalar.activation(offh[:, :], offb[:, :], Act.Copy,
                                 bias=-0.5, scale=1.0)
            # sigma_row4 = 4*rowp - 2 in {-2, +2}
            sigr = spool.tile([128, G * SD], BF16, tag="sigr")
            nc.scalar.activation(sigr[:, :], rowp[:, :], Act.Copy,
                                 bias=-2.0, scale=4.0)
            # sigma = sigr4 * offh in {+1, -1}, (b, j, ts) layout
            sig = spool.tile([128, G * SD], BF16, tag="sig")
            sig_e.tensor_tensor(
                out=sig[:, :],
                in0=sigr[:, :],
                in1=_ap(offh, [[J, G], [1, J], [0, TS]]),
                op=Alu.mult,
            )

            if stage < 6:
                out_e.dma_start(
                    out=of[b0:b0 + G, :].rearrange("b (p x) -> p b x", p=128),
                    in_=o[:, :],
                )
                return
            # out = q * sigma (broadcast over c), exact +/-1 multiply;
            # split by ts-range between VectorE and GpSimd, per clip
            tsplit = mult_split
            for b in range(G):
                if tsplit > 0:
                    nc.vector.tensor_tensor(
                        out=_ap(o, [[1, tsplit * JC]], extra_offset=b * FD),
                        in0=_ap(qt, [[1, tsplit * JC]], extra_offset=b * FD),
                        in1=_ap(sig, [[1, tsplit], [TS, J], [0, C]],
                                extra_offset=b * SD),
                        op=Alu.mult,
                    )
                if tsplit < TS:
                    nc.gpsimd.tensor_tensor(
                        out=_ap(o, [[1, (TS - tsplit) * JC]],
                                extra_offset=b * FD + tsplit * JC),
                        in0=_ap(qt, [[1, (TS - tsplit) * JC]],
                                extra_offset=b * FD + tsplit * JC),
                        in1=_ap(sig, [[1, TS - tsplit], [TS, J], [0, C]],
                                extra_offset=b * SD + tsplit),
                        op=Alu.mult,
                    )
                if out_split:
                    out_e.dma_start(
                        out=of[b0 + b, :].rearrange("(p x) -> p x", p=128),
                        in_=o[:, b * FD:(b + 1) * FD],
                    )

            if not out_split:
                out_e.dma_start(
                    out=of[b0:b0 + G, :].rearrange("b (p x) -> p b x", p=128),
                    in_=o[:, :],
                )

        if reps == 1:
            emit_body()
        else:
            with tc.For_i(0, reps, 1):
                emit_body()
    return nc


def make_consts():
    smat = np.eye(128, k=1, dtype=np.float32)       # S[k, m] = 1 iff m == k+1
    pmat = np.triu(np.ones((128, 128), np.float32), k=1)  # strict prefix
    return smat, pmat


def kernel(joint_rotations: np.ndarray) -> np.ndarray:
    q = np.ascontiguousarray(joint_rotations, dtype=np.float32)
    assert q.shape == (B, T, J, C)
    smat, pmat = make_consts()
    nc = build_nc()
    nc.finalize()   # run bacc passes (wait splitting, reg alloc) + freeze
    in_maps = [
        {"q": q[c * BPC:(c + 1) * BPC], "smat": smat, "pmat": pmat}
        for c in range(NCORES)
    ]
    res = run_bass_kernel_spmd(nc, in_maps, list(range(NCORES)))
    outs = [np.asarray(r["out"]) for r in res.results]
    return np.concatenate(outs, axis=0)


# revision 23
# speedup vs baseline: 24.6772x; 1.0537x over previous
"""Trainium2 Bass kernel for BatchRemoveQuatDiscontinuities.

Algorithm (per (batch, joint) lane):
    d[t]    = dot(q[t], q[t-1])                (fp32, 4-wide dot)
    flip[t] = 1 if d[t] < 0 else 0             (t >= 1; flip[0] = 0)
    sigma[t] = (-1)^(sum_{s<=t} flip[s])       (cumulative sign parity)
    out[t]  = q[t] * sigma[t]

Mapping on a NeuronCore (data-parallel over batch across 8 cores):
  * One tile = G batch clips ("gclips"), loaded as a single DMA:
    [128 partitions = t/8, free = (b: G, ts: 8, j: 64, c: 4)].  Each
    partition line is G contiguous-8KB spans, so DMA runs at the HBM
    roofline; G > 1 halves the instruction / semaphore count per clip
    (the kernel is latency- not throughput-bound).
  * q[t-1]: within a partition a free-axis offset (-256); the octet
    boundary (ts=0) comes from a TensorE shift-matmul into PSUM.
  * d via two pairwise adds; flip indicator e = Relu(Sign(-d)) on
    ScalarE (bf16); segmented xor-scan gives within-octet parity rowp;
    a strict-upper-triangular matmul counts odd octets above; parity
    of the count via int32 cast + bitwise-and; sigma = (4*rowp - 2) *
    (parity - 0.5) in {+/-1} exactly.
  * out = q * sigma (broadcast over c), exact +/-1 multiply on VectorE
    (GpSimd measured ~2.6 ns/elem vs DVE 1.16 on real HW and poisons the
    chain tail; mult_split=8 keeps it off the pipeline entirely).
  * Software-pipelined emission (prefetch=7): all in-DMAs are issued
    at the top of the loop body on the SP HWDGE queue (buffer-limited
    sliding prefetch), while out-DMAs issue from the Activation HWDGE
    queue — the two streams can never head-of-line block each other,
    and the For_i loop boundary doesn't drain the load pipeline.
    Per-clip out-DMAs (out_split) let the first MB stream out while
    the second clip's multiply finishes.
"""

import numpy as np
import ml_dtypes
from contextlib import ExitStack

import concourse.bass as bass
import concourse.bacc as bacc
import concourse.tile as tile
from concourse import mybir
from concourse.bass_utils import run_bass_kernel_spmd

B, T, J, C = 128, 1024, 64, 4
NCORES = 8
JC = J * C                      # 256 floats per t
BPC = B // NCORES               # 16 batch clips per core
TS = 8                          # t per partition (octet)
FD = TS * JC                    # per-clip free dim = 2048 floats
SD = J * TS                     # per-clip prefix free dim = 512 (j, ts)

FP32 = mybir.dt.float32
BF16 = mybir.dt.bfloat16
I32 = mybir.dt.int32
Alu = mybir.AluOpType
Act = mybir.ActivationFunctionType


def _ap(apx, dims, extra_offset=0):
    """AP with explicit [step, count] free dims appended to partition dim."""
    return bass.AP(
        tensor=apx.tensor, offset=apx.offset + extra_offset,
        ap=[list(apx.ap[0]), *[list(d) for d in dims]],
    )


def build_nc(bpc=BPC, t=T, reps=1, mode="full", mult_split=8,
             u_eng="vector", sig_eng="vector", out_q="scalar", in_q="sync",
             gclips=2, qbufs=6, obufs=3, sbufs=2, stage=99, prefetch=7,
             out_split=True, fixup=False, fuse_parity=False,
             scan_eng="vector"):
    assert t % (128 * TS) == 0
    G = gclips
    assert bpc % G == 0
    nc = bacc.Bacc(None, target_bir_lowering=False)
    q = nc.declare_dram_parameter("q", [bpc, t, J, C], FP32, isOutput=False)
    smat = nc.declare_dram_parameter("smat", [128, 128], FP32, isOutput=False)
    pmat = nc.declare_dram_parameter("pmat", [128, 128], FP32, isOutput=False)
    out = nc.declare_dram_parameter("out", [bpc, t, J, C], FP32, isOutput=True)
    qf = q.rearrange("b t j c -> b (t j c)")
    of = out.rearrange("b t j c -> b (t j c)")

    with tile.TileContext(nc) as tc, ExitStack() as ctx:
        consts = ctx.enter_context(tc.tile_pool(name="consts", bufs=1))
        qpool = ctx.enter_context(tc.tile_pool(name="qpool", bufs=qbufs))
        opool = ctx.enter_context(tc.tile_pool(name="opool", bufs=obufs))
        spool = ctx.enter_context(tc.tile_pool(name="spool", bufs=sbufs))
        auxp = ctx.enter_context(tc.tile_pool(name="auxp", bufs=4, space="PSUM"))
        offp = ctx.enter_context(tc.tile_pool(name="offp", bufs=4, space="PSUM"))

        smatSB = consts.tile([128, 128], FP32)
        nc.sync.dma_start(out=smatSB[:, :], in_=smat[:, :])
        pmatSB = consts.tile([128, 128], FP32)
        nc.sync.dma_start(out=pmatSB[:, :], in_=pmat[:, :])
        amask = consts.tile([128, G * SD], FP32)
        nc.vector.memset(amask[:, :], 1.0)
        nc.vector.memset(
            amask.rearrange("p (g j ts) -> p (g j) ts", j=J, ts=TS)[:, :, 0], 0.0
        )

        u_e = getattr(nc, u_eng)
        sig_e = getattr(nc, sig_eng)
        out_e = getattr(nc, out_q)
        in_e = getattr(nc, in_q)

        nchunks = bpc // G
        qts = {}

        def emit_load(g):
            b0 = g * G
            qts[g] = qpool.tile([128, G * FD], FP32, tag="qt", name=f"qt{g}")
            in_e.dma_start(
                out=qts[g][:, :],
                in_=qf[b0:b0 + G, :].rearrange("b (p x) -> p b x", p=128),
            )

        def emit_body():
            pf = min(prefetch, nchunks - 1)
            for g in range(pf):
                emit_load(g)
            for g in range(nchunks):
                if g + pf < nchunks:
                    emit_load(g + pf)
                emit_tile(g)

        def emit_tile(g):
            b0 = g * G
            qt = qts.pop(g)
            o = opool.tile([128, G * FD], FP32, tag="o")
            if mode == "dma":
                out_e.dma_start(
                    out=of[b0:b0 + G, :].rearrange("b (p x) -> p b x", p=128),
                    in_=qt[:, :],
                )
                return
            if mode.startswith("bench"):
                # mode = "bench:<eng>:<elems>:<count>" — rate microbench
                _, eng, elems, count = mode.split(":")
                elems, count = int(elems), int(count)
                be = getattr(nc, eng)
                for i in range(count):
                    if eng == "scalar":
                        be.activation(o[:, 0:elems], qt[:, 0:elems], Act.Sign,
                                      scale=-1.0)
                    else:
                        be.tensor_tensor(out=o[:, 0:elems], in0=qt[:, 0:elems],
                                         in1=qt[:, G * FD - elems:G * FD],
                                         op=Alu.mult)
                out_e.dma_start(
                    out=of[b0:b0 + G, :].rearrange("b (p x) -> p b x", p=128),
                    in_=o[:, :],
                )
                return
            if mode.startswith("bcast"):
                # mode = "bcast:<eng>:<count>" — broadcast-read microbench:
                # out 2048 contiguous = qt 2048 * qt[(ts,j)] bcast over C
                _, eng, count = mode.split(":")
                be = getattr(nc, eng)
                bc = _ap(qt, [[1, TS], [TS, J], [0, C]])
                for i in range(int(count)):
                    if eng == "scalar":
                        be.activation(o[:, 0:FD], bc, Act.Copy)
                    else:
                        be.tensor_tensor(out=o[:, 0:FD], in0=qt[:, 0:FD],
                                         in1=bc, op=Alu.mult)
                out_e.dma_start(
                    out=of[b0:b0 + G, :].rearrange("b (p x) -> p b x", p=128),
                    in_=o[:, :],
                )
                return

            # octet-boundary shift: aux[p, (b, jc)] = qt[p-1, b, ts=7 chunk]
            aux = auxp.tile([128, G * JC], FP32, tag="aux")
            nc.tensor.matmul(
                aux[:, :],
                lhsT=smatSB[:, :],
                rhs=_ap(qt, [[FD, G], [1, JC]], extra_offset=FD - JC),
                start=True,
                stop=True,
            )

            # prod: o = q * q_shifted  (DVE; the low chunk reads PSUM)
            nc.vector.tensor_tensor(
                out=_ap(o, [[FD, G], [1, FD - JC]], extra_offset=JC),
                in0=_ap(qt, [[FD, G], [1, FD - JC]], extra_offset=JC),
                in1=_ap(qt, [[FD, G], [1, FD - JC]]),
                op=Alu.mult,
            )
            nc.vector.tensor_tensor(
                out=_ap(o, [[FD, G], [1, JC]]),
                in0=_ap(qt, [[FD, G], [1, JC]]),
                in1=aux[:, :],
                op=Alu.mult,
            )

            if stage < 2:
                out_e.dma_start(
                    out=of[b0:b0 + G, :].rearrange("b (p x) -> p b x", p=128),
                    in_=o[:, :],
                )
                return
            # pairwise sums u = (c0+c1, c2+c3), (b, ts*j, k) order
            u = spool.tile([128, G * 2 * SD], FP32, tag="u")
            u_e.tensor_tensor(
                out=_ap(u, [[2 * SD, G], [2, SD], [1, 2]]),
                in0=_ap(o, [[FD, G], [4, SD], [2, 2]]),
                in1=_ap(o, [[FD, G], [4, SD], [2, 2]], extra_offset=1),
                op=Alu.add,
            )
            # d = u0 + u1, written in (b, j, ts) order for the scan
            d = spool.tile([128, G * SD], FP32, tag="d")
            nc.vector.tensor_tensor(
                out=_ap(d, [[SD, G], [1, TS], [TS, J]]),
                in0=_ap(u, [[2 * SD, G], [2 * J, TS], [2, J]]),
                in1=_ap(u, [[2 * SD, G], [2 * J, TS], [2, J]], extra_offset=1),
                op=Alu.add,
            )
            if stage < 3:
                out_e.dma_start(
                    out=of[b0:b0 + G, :].rearrange("b (p x) -> p b x", p=128),
                    in_=o[:, :],
                )
                return
            # flip indicator e = Relu(Sign(-d)), bf16
            sg = spool.tile([128, G * SD], FP32, tag="sg")
            nc.scalar.activation(sg[:, :], d[:, :], Act.Sign, scale=-1.0)
            e = spool.tile([128, G * SD], BF16, tag="e")
            nc.scalar.activation(e[:, :], sg[:, :], Act.Relu)
            if fixup:
                # t=0 has no flip (also guards Sign(0) semantics)
                nc.scalar.mul(
                    _ap(e[0:1, :], [[TS, G * J]]),
                    _ap(e[0:1, :], [[TS, G * J]]),
                    0.0,
                )

            if stage < 4:
                out_e.dma_start(
                    out=of[b0:b0 + G, :].rearrange("b (p x) -> p b x", p=128),
                    in_=o[:, :],
                )
                return
            # within-octet inclusive prefix PARITY (segmented xor-scan):
            # state = (mask * state) xor e  -> 0/1 running parity per joint
            rowp = spool.tile([128, G * SD], FP32, tag="rowp")
            getattr(nc, scan_eng).tensor_tensor_scan(
                out=rowp[:, :], data0=amask[:, :], data1=e[:, :],
                initial=0.0, op0=Alu.mult, op1=Alu.logical_xor,
            )

            if stage < 5:
                out_e.dma_start(
                    out=of[b0:b0 + G, :].rearrange("b (p x) -> p b x", p=128),
                    in_=o[:, :],
                )
                return
            # octet-level: count of odd rows above (parity-sum via matmul)
            offs = offp.tile([128, G * J], FP32, tag="offs")
            nc.tensor.matmul(
                offs[:, :],
                lhsT=pmatSB[:, :],
                rhs=_ap(rowp, [[SD, G], [TS, J]], extra_offset=TS - 1),
                start=True,
                stop=True,
            )
            # parity of the count: int cast + bitwise and
            offb = spool.tile([128, G * J], I32, tag="offb")
            if fuse_parity:
                nc.vector.tensor_scalar(
                    out=offb[:, :], in0=offs[:, :], scalar1=1, scalar2=None,
                    op0=Alu.bitwise_and,
                )
            else:
                offi = spool.tile([128, G * J], I32, tag="offi")
                nc.vector.tensor_copy(out=offi[:, :], in_=offs[:, :])
                nc.vector.tensor_scalar(
                    out=offb[:, :], in0=offi[:, :], scalar1=1, scalar2=None,
                    op0=Alu.bitwise_and,
                )
            # offh = parity - 0.5 in {-0.5, +0.5}
            offh = spool.tile([128, G * J], BF16, tag="offh")
            nc.scalar.activation(offh[:, :], offb[:, :], Act.Copy,
                                 bias=-0.5, scale=1.0)
            # sigma_row4 = 4*rowp - 2 in {-2, +2}
            sigr = spool.tile([128, G * SD], BF16, tag="sigr")
            nc.scalar.activation(sigr[:, :], rowp[:, :], Act.Copy,
                                 bias=-2.0, scale=4.0)
            # sigma = sigr4 * offh in {+1, -1}, (b, j, ts) layout
            sig = spool.tile([128, G * SD], BF16, tag="sig")
            sig_e.tensor_tensor(
                out=sig[:, :],
                in0=sigr[:, :],
                in1=_ap(offh, [[J, G], [1, J], [0, TS]]),
                op=Alu.mult,
            )

            if stage < 6:
                out_e.dma_start(
                    out=of[b0:b0 + G, :].rearrange("b (p x) -> p b x", p=128),
                    in_=o[:, :],
                )
                return
            # out = q * sigma (broadcast over c), exact +/-1 multiply;
            # split by ts-range between VectorE and GpSimd, per clip
            tsplit = mult_split
            for b in range(G):
                if tsplit > 0:
                    nc.vector.tensor_tensor(
                        out=_ap(o, [[1, tsplit * JC]], extra_offset=b * FD),
                        in0=_ap(qt, [[1, tsplit * JC]], extra_offset=b * FD),
                        in1=_ap(sig, [[1, tsplit], [TS, J], [0, C]],
                                extra_offset=b * SD),
                        op=Alu.mult,
                    )
                if tsplit < TS:
                    nc.gpsimd.tensor_tensor(
                        out=_ap(o, [[1, (TS - tsplit) * JC]],
                                extra_offset=b * FD + tsplit * JC),
                        in0=_ap(qt, [[1, (TS - tsplit) * JC]],
                                extra_offset=b * FD + tsplit * JC),
                        in1=_ap(sig, [[1, TS - tsplit], [TS, J], [0, C]],
                                extra_offset=b * SD + tsplit),
                        op=Alu.mult,
                    )
                if out_split:
                    out_e.dma_start(
                        out=of[b0 + b, :].rearrange("(p x) -> p x", p=128),
                        in_=o[:, b * FD:(b + 1) * FD],
                    )

            if not out_split:
                out_e.dma_start(
                    out=of[b0:b0 + G, :].rearrange("b (p x) -> p b x", p=128),
                    in_=o[:, :],
                )

        if reps == 1:
            emit_body()
        else:
            with tc.For_i(0, reps, 1):
                emit_body()
    return nc


def make_consts():
    smat = np.eye(128, k=1, dtype=np.float32)       # S[k, m] = 1 iff m == k+1
    pmat = np.triu(np.ones((128, 128), np.float32), k=1)  # strict prefix
    return smat, pmat


def kernel(joint_rotations: np.ndarray) -> np.ndarray:
    q = np.ascontiguousarray(joint_rotations, dtype=np.float32)
    assert q.shape == (B, T, J, C)
    smat, pmat = make_consts()
    nc = build_nc()
    nc.finalize()   # run bacc passes (wait splitting, reg alloc) + freeze
    in_maps = [
        {"q": q[c * BPC:(c + 1) * BPC], "smat": smat, "pmat": pmat}
        for c in range(NCORES)
    ]
    res = run_bass_kernel_spmd(nc, in_maps, list(range(NCORES)))
    outs = [np.asarray(r["out"]) for r in res.results]
    return np.concatenate(outs, axis=0)
